# revision 4
# baseline (speedup 1.0000x reference)
"""Trainium2 Bass kernel for nn_MixtureLinear.

Math:  out[b,n,d] = sum_{c,r} input[b,n,c] * weight[d,c,r] * coef[n,r]
                    + sum_r coef[n,r] * bias[d,r]

Sharding: data-parallel over batch: core b computes batch b entirely
(B == 8 == n_cores). Per core that is a [N=1024 tokens] x [C=1024] x
[D=1024] x [R=8] problem = 17.2 GFLOP.

Per-core algorithm (all matmuls in float32r -> full PE rate at N=512):
  - xT [c, n] resident in SBUF (host pre-transposed; contraction dim on
    partitions, as the tensor engine requires for both operands).
  - for each d-halftile (512 wide), for each rank r:
      Y_r = xT.T @ Wt_r   accumulated over 8 c-tiles in one PSUM bank
      acc += coef[:, r] * Y_r   (DVE scalar_tensor_tensor; coef slice is a
                                 per-partition scalar over the token dim)
  - acc is initialized with the bias term coef @ bias.T computed by a tiny
    K=8 matmul (lhsT=coefT tile, rhs=biasT tile) + ScalarE PSUM->SBUF copy.
  - Wt (= weight rearranged to [r, c, d], host-side) streams from HBM,
    double-buffered; each tile is read exactly once.
"""

import sys

if "/opt/trn_rl_repo" not in sys.path:
    sys.path.insert(0, "/opt/trn_rl_repo")

import numpy as np

B, N, C, D, R = 8, 1024, 1024, 1024, 8
P = 128      # SBUF partitions
DTILE = 512  # matmul moving free dim (one fp32 PSUM bank)
N_CORES = 8

_CACHE = {}


def _build_nc(n=N, c=C, d=D, r=R):
    import concourse.mybir as mybir
    import concourse.tile as tile
    from concourse import bacc

    f32 = mybir.dt.float32
    f32r = mybir.dt.float32r
    mult = mybir.AluOpType.mult
    add = mybir.AluOpType.add

    KT = c // P       # contraction tiles
    MT = n // P       # token tiles
    DT = d // DTILE   # output free-dim tiles

    nc = bacc.Bacc()
    xt = nc.dram_tensor("xt", [c, n], f32r, kind="ExternalInput")
    wt = nc.dram_tensor("wt", [r * c, d], f32r, kind="ExternalInput")
    coef = nc.dram_tensor("coef", [n, r], f32, kind="ExternalInput")
    coefT = nc.dram_tensor("coefT", [r, n], f32r, kind="ExternalInput")
    biasT = nc.dram_tensor("biasT", [r, d], f32r, kind="ExternalInput")
    out = nc.dram_tensor("out", [n, d], f32, kind="ExternalOutput")

    with tile.TileContext(nc) as tc:
        with (
            tc.tile_pool(name="consts", bufs=1) as cpool,
            tc.tile_pool(name="wpool", bufs=16) as wpool,
            tc.tile_pool(name="accpool", bufs=2 * MT) as apool,
            tc.tile_pool(name="psum", bufs=1, space="PSUM") as pspool,
        ):
            xt_sb = []
            for k in range(KT):
                t = cpool.tile([P, n], f32r, name=f"xt_sb{k}", tag=f"xt_sb{k}")
                nc.sync.dma_start(t, xt[k * P : (k + 1) * P, :])
                xt_sb.append(t)
            coef_sb = []
            for m in range(MT):
                t = cpool.tile([P, r], f32, name=f"coef_sb{m}", tag=f"coef_sb{m}")
                nc.sync.dma_start(t, coef[m * P : (m + 1) * P, :])
                coef_sb.append(t)
            coefT_sb = cpool.tile([r, n], f32r, name="coefT_sb", tag="coefT_sb")
            nc.sync.dma_start(coefT_sb, coefT[:, :])
            biasT_sb = cpool.tile([r, d], f32r, name="biasT_sb", tag="biasT_sb")
            nc.sync.dma_start(biasT_sb, biasT[:, :])

            for dt in range(DT):
                dsl = slice(dt * DTILE, (dt + 1) * DTILE)
                accs = []
                for m in range(MT):
                    bias_ps = pspool.tile(
                        [P, DTILE], f32, name="bias_ps", tag="bias_ps", bufs=2
                    )
                    nc.tensor.matmul(
                        bias_ps,
                        coefT_sb[:, m * P : (m + 1) * P],
                        biasT_sb[:, dsl],
                        start=True,
                        stop=True,
                    )
                    acc = apool.tile([P, DTILE], f32, name=f"acc{m}", tag="acc")
                    nc.scalar.copy(acc, bias_ps)
                    accs.append(acc)
                for rr in range(r):
                    wts = []
                    for k in range(KT):
                        w = wpool.tile([P, DTILE], f32r, name="w", tag="w")
                        nc.sync.dma_start(
                            w, wt[rr * c + k * P : rr * c + (k + 1) * P, dsl]
                        )
                        wts.append(w)
                    for m in range(MT):
                        y = pspool.tile([P, DTILE], f32, name="y", tag="y", bufs=5)
                        for k in range(KT):
                            nc.tensor.matmul(
                                y,
                                xt_sb[k][:, m * P : (m + 1) * P],
                                wts[k],
                                start=(k == 0),
                                stop=(k == KT - 1),
                            )
                        nc.vector.scalar_tensor_tensor(
                            accs[m], y, coef_sb[m][:, rr : rr + 1], accs[m], mult, add
                        )
                for m in range(MT):
                    nc.sync.dma_start(out[m * P : (m + 1) * P, dsl], accs[m])
    nc.finalize()
    return nc


def _get_nc():
    if "nc" not in _CACHE:
        _CACHE["nc"] = _build_nc()
    return _CACHE["nc"]


def _prepare_in_maps(inputs):
    f32 = np.float32
    input_ = np.asarray(inputs["input"], dtype=f32)
    weight = np.asarray(inputs["weight"], dtype=f32)
    bias = np.asarray(inputs["bias"], dtype=f32)
    coef = np.asarray(inputs["coef"], dtype=f32)

    wt = np.ascontiguousarray(weight.transpose(2, 1, 0)).reshape(R * C, D)
    coefT = np.ascontiguousarray(coef.T)
    biasT = np.ascontiguousarray(bias.T)
    coef_c = np.ascontiguousarray(coef)

    in_maps = []
    for b in range(B):
        in_maps.append(
            {
                "xt": np.ascontiguousarray(input_[b].T),
                "wt": wt,
                "coef": coef_c,
                "coefT": coefT,
                "biasT": biasT,
            }
        )
    return in_maps


def _install_ntff_hook_shim():
    """The agent image lacks antenv.axon_hooks; recreate it from the ctypes
    hook factory in trn_agent_boot so trace=True can capture NTFF profiles."""
    import types

    if "antenv.axon_hooks" in sys.modules:
        return
    try:
        from trn_agent_boot.trn_boot import _ntff_profile_via_ctypes

        hook = _ntff_profile_via_ctypes("/opt/axon/libaxon_pjrt.so")
        mod = types.ModuleType("antenv.axon_hooks")
        mod.get_axon_ntff_profile_hook = lambda: hook
        sys.modules["antenv.axon_hooks"] = mod
    except Exception as e:  # profiling is best-effort; execution still works
        print(f"ntff hook shim unavailable: {e}")


def _run(inputs, trace=False, **kwargs):
    from concourse.bass_utils import run_bass_kernel_spmd

    if trace:
        _install_ntff_hook_shim()
    in_maps = _prepare_in_maps(inputs)
    nc = _get_nc()
    res = run_bass_kernel_spmd(
        nc, in_maps, core_ids=list(range(N_CORES)), trace=trace, **kwargs
    )
    out = np.stack([r["out"] for r in res.results], axis=0)
    return out, res


def kernel(**inputs) -> np.ndarray:
    out, _ = _run(inputs)
    return out


# revision 5
# speedup vs baseline: 1.0313x; 1.0313x over previous
"""Trainium2 Bass kernel for nn_MixtureLinear.

Math:  out[b,n,d] = sum_{c,r} input[b,n,c] * weight[d,c,r] * coef[n,r]
                    + sum_r coef[n,r] * bias[d,r]

Sharding: data-parallel over batch: core b computes batch b entirely
(B == 8 == n_cores). Per core that is a [N=1024 tokens] x [C=1024] x
[D=1024] x [R=8] problem = 17.2 GFLOP.

Per-core algorithm (all matmuls in float32r -> full PE rate at N=512):
  - xT [c, n] resident in SBUF (host pre-transposed; contraction dim on
    partitions, as the tensor engine requires for both operands).
  - for each d-halftile (512 wide), for each rank r:
      Y_r = xT.T @ Wt_r   accumulated over 8 c-tiles in one PSUM bank
      acc += coef[:, r] * Y_r   (DVE scalar_tensor_tensor; coef slice is a
                                 per-partition scalar over the token dim)
  - acc is initialized with the bias term coef @ bias.T computed by a tiny
    K=8 matmul (lhsT=coefT tile, rhs=biasT tile) + ScalarE PSUM->SBUF copy.
  - Wt (= weight rearranged to [r, c, d], host-side) streams from HBM,
    double-buffered; each tile is read exactly once.
"""

import sys

if "/opt/trn_rl_repo" not in sys.path:
    sys.path.insert(0, "/opt/trn_rl_repo")

import numpy as np

B, N, C, D, R = 8, 1024, 1024, 1024, 8
P = 128      # SBUF partitions
DTILE = 512  # matmul moving free dim (one fp32 PSUM bank)
N_CORES = 8

_CACHE = {}


def _build_nc(n=N, c=C, d=D, r=R):
    import concourse.mybir as mybir
    import concourse.tile as tile
    from concourse import bacc

    f32 = mybir.dt.float32
    f32r = mybir.dt.float32r
    mult = mybir.AluOpType.mult
    add = mybir.AluOpType.add

    KT = c // P       # contraction tiles
    MT = n // P       # token tiles
    DT = d // DTILE   # output free-dim tiles

    nc = bacc.Bacc()
    xt = nc.dram_tensor("xt", [c, n], f32r, kind="ExternalInput")
    wt = nc.dram_tensor("wt", [r * c, d], f32r, kind="ExternalInput")
    coef = nc.dram_tensor("coef", [n, r], f32, kind="ExternalInput")
    coefT = nc.dram_tensor("coefT", [r, n], f32r, kind="ExternalInput")
    biasT = nc.dram_tensor("biasT", [r, d], f32r, kind="ExternalInput")
    out = nc.dram_tensor("out", [n, d], f32, kind="ExternalOutput")

    QT = 4 if n % (4 * P) == 0 else 1  # token-quarter split for xt loads
    QW = n // QT

    with tile.TileContext(nc) as tc:
        with (
            tc.tile_pool(name="consts", bufs=1) as cpool,
            tc.tile_pool(name="wpool", bufs=24) as wpool,
            tc.tile_pool(name="accpool", bufs=DT * MT) as apool,
            tc.tile_pool(name="psum", bufs=1, space="PSUM") as pspool,
        ):
            # tiny operands for the bias matmuls come first
            coefT_sb = cpool.tile([r, n], f32r, name="coefT_sb", tag="coefT_sb")
            nc.sync.dma_start(coefT_sb, coefT[:, :])
            biasT_sb = cpool.tile([r, d], f32r, name="biasT_sb", tag="biasT_sb")
            nc.sync.dma_start(biasT_sb, biasT[:, :])

            # all accumulators upfront, initialized with the bias term while
            # the big DMAs stream in (K=8 matmul + ScalarE PSUM->SBUF copy)
            accs = {}
            for dt in range(DT):
                dsl = slice(dt * DTILE, (dt + 1) * DTILE)
                for m in range(MT):
                    bias_ps = pspool.tile(
                        [P, DTILE], f32, name="bias_ps", tag="bias_ps", bufs=2
                    )
                    nc.tensor.matmul(
                        bias_ps,
                        coefT_sb[:, m * P : (m + 1) * P],
                        biasT_sb[:, dsl],
                        start=True,
                        stop=True,
                    )
                    acc = apool.tile([P, DTILE], f32, name=f"acc{dt}_{m}", tag="acc")
                    nc.scalar.copy(acc, bias_ps)
                    accs[dt, m] = acc

            xt_sb = [
                cpool.tile([P, n], f32r, name=f"xt_sb{k}", tag=f"xt_sb{k}")
                for k in range(KT)
            ]

            def load_xt_quarter(q):
                for k in range(KT):
                    nc.sync.dma_start(
                        xt_sb[k][:, q * QW : (q + 1) * QW],
                        xt[k * P : (k + 1) * P, q * QW : (q + 1) * QW],
                    )

            def load_w_group(dt, rr):
                dsl = slice(dt * DTILE, (dt + 1) * DTILE)
                wts = []
                for k in range(KT):
                    w = wpool.tile([P, DTILE], f32r, name="w", tag="w")
                    nc.sync.dma_start(
                        w, wt[rr * c + k * P : rr * c + (k + 1) * P, dsl]
                    )
                    wts.append(w)
                return wts

            # DMA emission order tuned so data arrives in consumption order:
            # first m-groups need xt quarter 0 + the first w group only.
            load_xt_quarter(0)
            w_groups = {}
            w_groups[0, 0] = load_w_group(0, 0)
            coef_sb = []
            for m in range(MT):
                t = cpool.tile([P, r], f32, name=f"coef_sb{m}", tag=f"coef_sb{m}")
                nc.sync.dma_start(t, coef[m * P : (m + 1) * P, :])
                coef_sb.append(t)
            if QT > 1:
                load_xt_quarter(1)
                load_xt_quarter(2)
            if r > 1:
                w_groups[0, 1] = load_w_group(0, 1)
            if QT > 1:
                load_xt_quarter(3)

            for dt in range(DT):
                for rr in range(r):
                    wts = w_groups.pop((dt, rr), None)
                    if wts is None:
                        wts = load_w_group(dt, rr)
                    for m in range(MT):
                        y = pspool.tile([P, DTILE], f32, name="y", tag="y", bufs=5)
                        for k in range(KT):
                            nc.tensor.matmul(
                                y,
                                xt_sb[k][:, m * P : (m + 1) * P],
                                wts[k],
                                start=(k == 0),
                                stop=(k == KT - 1),
                            )
                        nc.vector.scalar_tensor_tensor(
                            accs[dt, m],
                            y,
                            coef_sb[m][:, rr : rr + 1],
                            accs[dt, m],
                            mult,
                            add,
                        )
                dsl = slice(dt * DTILE, (dt + 1) * DTILE)
                for m in range(MT):
                    nc.sync.dma_start(out[m * P : (m + 1) * P, dsl], accs[dt, m])
    nc.finalize()
    return nc


def _get_nc():
    if "nc" not in _CACHE:
        _CACHE["nc"] = _build_nc()
    return _CACHE["nc"]


def _prepare_in_maps(inputs):
    f32 = np.float32
    input_ = np.asarray(inputs["input"], dtype=f32)
    weight = np.asarray(inputs["weight"], dtype=f32)
    bias = np.asarray(inputs["bias"], dtype=f32)
    coef = np.asarray(inputs["coef"], dtype=f32)

    wt = np.ascontiguousarray(weight.transpose(2, 1, 0)).reshape(R * C, D)
    coefT = np.ascontiguousarray(coef.T)
    biasT = np.ascontiguousarray(bias.T)
    coef_c = np.ascontiguousarray(coef)

    in_maps = []
    for b in range(B):
        in_maps.append(
            {
                "xt": np.ascontiguousarray(input_[b].T),
                "wt": wt,
                "coef": coef_c,
                "coefT": coefT,
                "biasT": biasT,
            }
        )
    return in_maps


def _install_ntff_hook_shim():
    """The agent image lacks antenv.axon_hooks; recreate it from the ctypes
    hook factory in trn_agent_boot so trace=True can capture NTFF profiles."""
    import types

    if "antenv.axon_hooks" in sys.modules:
        return
    try:
        from trn_agent_boot.trn_boot import _ntff_profile_via_ctypes

        hook = _ntff_profile_via_ctypes("/opt/axon/libaxon_pjrt.so")
        mod = types.ModuleType("antenv.axon_hooks")
        mod.get_axon_ntff_profile_hook = lambda: hook
        sys.modules["antenv.axon_hooks"] = mod
    except Exception as e:  # profiling is best-effort; execution still works
        print(f"ntff hook shim unavailable: {e}")


def _run(inputs, trace=False, **kwargs):
    from concourse.bass_utils import run_bass_kernel_spmd

    if trace:
        _install_ntff_hook_shim()
    in_maps = _prepare_in_maps(inputs)
    nc = _get_nc()
    res = run_bass_kernel_spmd(
        nc, in_maps, core_ids=list(range(N_CORES)), trace=trace, **kwargs
    )
    out = np.stack([r["out"] for r in res.results], axis=0)
    return out, res


def kernel(**inputs) -> np.ndarray:
    out, _ = _run(inputs)
    return out
